# revision 54
# baseline (speedup 1.0000x reference)
"""Trainium2 Bass kernel for nn_CausePredictor (RGCN + pairwise MLP). v2

Sharding: data-parallel over the pairwise row index i (dim 1 of the
[B,S,S] output): 8 cores x 25 rows, replicated over B=4.  All per-core
differences are encoded as input DATA, so one SPMD program serves all
cores.

Math (matching reference.py):
  h   = sum_k Ahat_k.T @ (x[b] @ basis_k) + x[b] @ root + bias
  u   = h @ W1a (j-term),  v = h @ W1c (i-term)
  T   = pe_k @ W1b + pe_v @ W1d            # [11, 512], computed on host
  peR[m, mc, (i,j)] = T[pos(i,j), mc*128+m]  # host-gathered, bf16
  h1[b,i,j,:] = u[b,j] + v[b,i] + T[pos(i,j)]
  out = sigmoid(relu(relu(h1) @ W2) @ Wp) * mask

v2 changes vs v1 (288us):
  - stage A matmuls moved off fp32 (4 cyc/row) to f32r/bf16 moving
    operands (1 cyc/row), batching 2 batches per psum where needed to
    reach the 256-wide f32r full-rate threshold.
  - T table + its [11 -> 5000] positional gather precomputed on host
    (frees ~9us of PE + copies).
  - GEMM3 (x @ Wp) flipped: rh2 is the stationary operand, Wp the
    1-column moving operand -> 1-cycle matmuls into [100,1] psum
    columns instead of 400-cycle [1,400] rows.
  - sigmoid batched per-b over [100, 50] instead of per-unit [1, 400].
  - relu2 split ACT/Pool; rh1 adds fold all 4 m-chunks into one DVE op.
"""

import sys

sys.path.insert(0, "/opt/trn_rl_repo")

import numpy as np

B, S, D, M, P = 4, 200, 300, 512, 100
NREL, MAXL = 9, 10
NCORES = 8
IPC = S // NCORES  # 25 rows of i per core
SC = S + IPC  # 225: all-j columns + the core's i-slice
FPC = IPC * S  # 5000 pairs per (b, core)
NU = IPC // 2 + 1  # 13 units per b: 12x 2-row + 1x 1-row
NCH = FPC // 100  # 50 GEMM3 chunks of 100 pairs per b

DCW = [128, 128, 44]  # D=300 chunks
JCW = [128, 72]  # S=200 chunks

_prog_cache = {}
NWARM = 48


def _rel_adj(s):
    ra = np.arange(s)[None, :] - np.arange(s)[:, None]
    for i in range(s):
        ra[i, i + 1 :] = 1
        num = 1
        for o in range(i - 1, -1, -2):
            ra[i, o] = -num
            if o - 1 >= 0:
                ra[i, o - 1] = -num
            num += 1
        ra[i, :i] = np.maximum(ra[i, :i], -8)  # -(WINDOW+1), WINDOW=7
    return ra


def _pack_k(w):
    """[K, N] -> [128, ceil(K/128)*N], K chunked onto partitions, zero pad."""
    k, n = w.shape
    nch = (k + 127) // 128
    out = np.zeros((128, nch * n), np.float32)
    for c in range(nch):
        r = min(128, k - c * 128)
        out[:r, c * n : c * n + n] = w[c * 128 : c * 128 + r]
    return out


def _build_program():
    import ml_dtypes  # noqa: F401
    import concourse.tile as tile
    from concourse import bacc, mybir

    f32 = mybir.dt.float32
    f32r = mybir.dt.float32r
    bf16 = mybir.dt.bfloat16
    AF = mybir.ActivationFunctionType
    OP = mybir.AluOpType

    nc = bacc.Bacc()

    dxT = nc.declare_dram_parameter("xT", [D, B * SC], bf16, isOutput=False)
    dahat = nc.declare_dram_parameter("ahat", [128, 4 * SC], bf16, isOutput=False)
    dbasis = nc.declare_dram_parameter("basis", [128, 1800], bf16, isOutput=False)
    droot = nc.declare_dram_parameter("root", [128, 900], bf16, isOutput=False)
    dbias = nc.declare_dram_parameter("bias", [128, 3], f32, isOutput=False)
    dw1a = nc.declare_dram_parameter("w1a", [128, 1536], bf16, isOutput=False)
    dw1c = nc.declare_dram_parameter("w1c", [128, 1536], bf16, isOutput=False)
    dw2 = nc.declare_dram_parameter("w2", [128, 4 * M], bf16, isOutput=False)
    dwp = nc.declare_dram_parameter("wp", [128, 4], bf16, isOutput=False)
    dpeR = nc.declare_dram_parameter("peR", [128, NU * 1600], bf16, isOutput=False)
    dout = nc.declare_dram_parameter("out", [100, B * NCH], f32, isOutput=True)

    with tile.TileContext(nc) as tc:
        with (
            tc.tile_pool(name="persist", bufs=1) as pp,
            tc.tile_pool(name="work", bufs=5) as pwork,
            tc.tile_pool(name="sigp", bufs=2) as psig,
        ):
            # Dummy sigmoid up front so the activation-table load (the set
            # containing sigmoid covers identity/copy/relu too) happens during
            # the DMA head instead of mid-kernel at the first real sigmoid.
            scratch = pp.tile([1, 2], f32, tag="scratch", name="scratch")
            nc.vector.memzero(scratch[:, :])
            nc.scalar.activation(scratch[:, :], scratch[:, :], AF.Sigmoid)
            # Warm up the PE p-state during the DMA head: dummy matmuls keep
            # pe_busy_start early so real stage-A matmuls run at full clock.
            warm_sb = pp.tile([1, 64], bf16, tag="warm_sb", name="warm_sb")
            nc.vector.memzero(warm_sb[:, :])

            # ---------------- input loads (need-ordered) -----------------
            # interleave xT/basis chunk loads so the first t1 group can
            # start as early as possible
            basis = pp.tile([128, 1800], bf16, tag="basis", name="basis")
            xT = [
                pp.tile([DCW[c], B, SC], bf16, tag=f"xT{c}", name=f"xT{c}")
                for c in range(3)
            ]
            for c in range(3):
                nc.sync.dma_start(xT[c][:, :, :], dxT[c * 128 : c * 128 + DCW[c], :])
                nc.sync.dma_start(basis[:, c * D : (c + 1) * D],
                                  dbasis[:, c * D : (c + 1) * D])
            for dc in range(3):
                nc.sync.dma_start(basis[:, 900 + dc * D : 900 + (dc + 1) * D],
                                  dbasis[:, 900 + dc * D : 900 + (dc + 1) * D])

            def load2d(name, shape, dt, src):
                t = pp.tile(shape, dt, tag=name, name=name)
                nc.sync.dma_start(t[:, :], src[:, :])
                return t

            ahat = load2d("ahat", [128, 4 * SC], bf16, dahat)
            root = load2d("root", [128, 900], bf16, droot)
            bias = load2d("bias", [128, 3], f32, dbias)

            w1a = pp.tile([128, 3, M], bf16, tag="w1a", name="w1a")
            w1c = pp.tile([128, 3, M], bf16, tag="w1c", name="w1c")
            nc.sync.dma_start(w1a[:, :, :], dw1a[:, :])
            nc.sync.dma_start(w1c[:, :, :], dw1c[:, :])

            w2 = pp.tile([128, 4, M], bf16, tag="w2", name="w2")
            nc.sync.dma_start(w2[:, :, :], dw2[:, :])
            wp = load2d("wp", [128, 4], bf16, dwp)

            # peR in 13 unit-blocks, u-major, so stage B's first units load first
            peR = pp.tile([128, NU, 4, 400], bf16, tag="peR", name="peR")
            for u in range(NU):
                nc.sync.dma_start(peR[:, u, :, :], dpeR[:, u * 1600 : (u + 1) * 1600])

            # ---------------- persistent activations --------------------
            t1 = [
                [
                    [
                        pp.tile([JCW[jc], D], bf16, tag=f"t1_{b}{k}{jc}",
                                name=f"t1_{b}{k}{jc}")
                        for jc in range(2)
                    ]
                    for k in range(2)
                ]
                for b in range(B)
            ]
            hTu = [
                pp.tile([DCW[ec], B, S], bf16, tag=f"hTu{ec}", name=f"hTu{ec}")
                for ec in range(3)
            ]
            hTv = [
                pp.tile([DCW[ec], B, IPC], bf16, tag=f"hTv{ec}", name=f"hTv{ec}")
                for ec in range(3)
            ]
            uu = [
                pp.tile([128, 4, 2 * S], bf16, tag=f"uu{bp}", name=f"uu{bp}")
                for bp in range(2)
            ]
            v4 = pp.tile([128, 4, B * IPC], f32, tag="v4", name="v4")

            def emit_rh1(b, u, no_act=False):
                """Build relu-less h1 inputs: rh1 = (u_j + peR) then (+v_i, relu)."""
                bp, half = divmod(b, 2)
                nil = 2 if u < NU - 1 else 1
                rh1 = pwork.tile([128, 4, 2 * S], bf16, tag="rh1", name="rh1")
                for h in range(nil):
                    nc.vector.tensor_add(
                        rh1[:, :, h * S : (h + 1) * S],
                        uu[bp][:, :, half * S : (half + 1) * S],
                        peR[:, u, :, h * S : h * S + S],
                    )
                idx = 0
                for mc in range(4):
                    for h in range(nil):
                        sl = (slice(None), mc, slice(h * S, h * S + S))
                        vsl = v4[:, mc, b * IPC + 2 * u + h : b * IPC + 2 * u + h + 1]
                        if idx in (0, 4) and not no_act:
                            nc.scalar.activation(rh1[sl], rh1[sl], AF.Relu, bias=vsl)
                        elif idx in (2,) or (no_act and idx in (0, 4)):
                            nc.vector.tensor_scalar(
                                out=rh1[sl], in0=rh1[sl], scalar1=vsl,
                                scalar2=0.0, op0=OP.add, op1=OP.max)
                        else:
                            nc.gpsimd.tensor_scalar(
                                out=rh1[sl], in0=rh1[sl], scalar1=vsl,
                                scalar2=0.0, op0=OP.add, op1=OP.max)
                        idx += 1
                return rh1

            # ---------------- stage A -----------------------------------
            with (
                tc.tile_pool(name="psT1", bufs=3, space="PSUM") as psT1,
                tc.tile_pool(name="psH", bufs=2, space="PSUM") as psH,
                tc.tile_pool(name="psU", bufs=3, space="PSUM") as psU,
            ):
                warm_ps = psT1.tile([1, 64], f32, tag="t1ps", name="warm_ps")

                def warm(n):
                    for _ in range(n):
                        nc.tensor.matmul(
                            warm_ps[:, :], warm_sb[:, 0:1], warm_sb[:, :],
                            start=True, stop=True,
                        )

                warm(NWARM)
                # t1[b,k] = x_b @ basis_k   (k-outer so k1 DMA overlaps k0 work)
                nv = 0
                for k in range(2):
                    for b in range(B):
                        for jc in range(2):
                            t1ps = psT1.tile([JCW[jc], D], f32, tag="t1ps", name="t1ps")
                            for dc in range(3):
                                nc.tensor.matmul(
                                    t1ps[:, :],
                                    xT[dc][:, b, jc * 128 : jc * 128 + JCW[jc]],
                                    basis[0 : DCW[dc], (k * 3 + dc) * D : (k * 3 + dc + 1) * D],
                                    start=(dc == 0),
                                    stop=(dc == 2),
                                )
                            if nv % 2 == 0:
                                nc.vector.tensor_copy(t1[b][k][jc][:, :], t1ps[:, :])
                            else:
                                nc.scalar.activation(t1[b][k][jc][:, :], t1ps[:, :],
                                                     AF.Copy)
                            nv += 1
                            if nv <= 8:
                                warm(6)

                # h = sum_k Ahat_k.T @ t1 + x @ root + bias, per 2-batch pair
                def emit_h(bp):
                    for ec in range(3):
                        hps = psH.tile([DCW[ec], 2, SC], f32, tag="hps", name="hps")
                        for dc in range(3):
                            nc.tensor.matmul(
                                hps[:, :, :],
                                root[0 : DCW[dc], dc * D + ec * 128 : dc * D + ec * 128 + DCW[ec]],
                                xT[dc][:, 2 * bp : 2 * bp + 2, :],
                                start=(dc == 0),
                                stop=False,
                            )
                        for half in range(2):
                            b = 2 * bp + half
                            for k in range(2):
                                for jc in range(2):
                                    nc.tensor.matmul(
                                        hps[:, half, :],
                                        t1[b][k][jc][:, ec * 128 : ec * 128 + DCW[ec]],
                                        ahat[0 : JCW[jc], (k * 2 + jc) * SC : (k * 2 + jc + 1) * SC],
                                        start=False,
                                        stop=(half == 1 and k == 1 and jc == 1),
                                    )
                        if bp == 0:
                            nc.vector.tensor_scalar(
                                out=hTv[ec][:, 2 * bp : 2 * bp + 2, :],
                                in0=hps[:, :, S:SC],
                                scalar1=bias[0 : DCW[ec], ec : ec + 1],
                                scalar2=None,
                                op0=OP.add,
                            )
                        else:
                            # hTv first: v(bp1) consumes it before u(bp1)
                            # needs hTu, and both queue on ACT in order.
                            nc.scalar.activation(
                                hTv[ec][:, 2 * bp : 2 * bp + 2, :],
                                hps[:, :, S:SC],
                                AF.Identity,
                                bias=bias[0 : DCW[ec], ec : ec + 1],
                            )
                        nc.scalar.activation(
                            hTu[ec][:, 2 * bp : 2 * bp + 2, :],
                            hps[:, :, 0:S],
                            AF.Identity,
                            bias=bias[0 : DCW[ec], ec : ec + 1],
                        )

                def emit_u(bp, pool):
                    for mc in range(4):
                        ups = pool.tile([128, 2 * S], f32, tag="ops", name="ups")
                        for ec in range(3):
                            nc.tensor.matmul(
                                ups[:, :],
                                w1a[0 : DCW[ec], ec, mc * 128 : mc * 128 + 128],
                                hTu[ec][:, 2 * bp : 2 * bp + 2, 0:S],
                                start=(ec == 0),
                                stop=(ec == 2),
                            )
                        if mc % 2 == 0:
                            nc.vector.tensor_copy(uu[bp][:, mc, :], ups[:, :])
                        else:
                            nc.scalar.activation(uu[bp][:, mc, :], ups[:, :], AF.Copy)

                def emit_v(bp, pool):
                    # v = h_i @ W1c for one 2-batch pair (i-slice cols of hTv)
                    for mc in range(4):
                        vps = pool.tile([128, 2 * IPC], f32, tag="ops", name="vps")
                        for ec in range(3):
                            nc.tensor.matmul(
                                vps[:, :],
                                w1c[0 : DCW[ec], ec, mc * 128 : mc * 128 + 128],
                                hTv[ec][:, 2 * bp : 2 * bp + 2, :],
                                start=(ec == 0),
                                stop=(ec == 2),
                            )
                        if bp == 0:
                            nc.vector.tensor_copy(
                                v4[:, mc, 2 * bp * IPC : (2 * bp + 2) * IPC], vps[:, :])
                        else:
                            nc.scalar.activation(
                                v4[:, mc, 2 * bp * IPC : (2 * bp + 2) * IPC],
                                vps[:, :], AF.Identity)

                emit_h(0)
                emit_v(0, psU)
                emit_u(0, psU)
                # prebuild rh1 for (b0, u0..u2): vector engines fill while PE
                # runs h(bp1); the first two stage-B units then interleave
                # before v/u(bp1) so PE never idles at the transition.
                emit_h(1)
                prebuilt = {}
                for u in range(4):
                    prebuilt[(0, u)] = emit_rh1(0, u, no_act=True)
                emit_v(1, psU)
                emit_u(1, psU)

            # ---------------- stage B: pairwise MLP ----------------------
            with (
                tc.tile_pool(name="ps2", bufs=5, space="PSUM") as ps2,
                tc.tile_pool(name="ps3", bufs=2, space="PSUM") as ps3,
            ):
                sgouts = {}

                def emit_unit(b, u):
                    bp, half = divmod(b, 2)
                    if u == 0:
                        sgouts[b] = psig.tile([100, NCH], f32, tag="sgout",
                                              name="sgout")
                    sgout = sgouts[b]
                    if True:
                        nil = 2 if u < NU - 1 else 1
                        ncols = nil * S
                        rh1 = prebuilt.pop((b, u), None)
                        if rh1 is None:
                            rh1 = emit_rh1(b, u)
                        # GEMM2 + relu2 (ACT for n<2, Pool for n>=2)
                        rh2 = pwork.tile([128, 4, 2 * S], bf16, tag="rh2", name="rh2")
                        for n in range(4):
                            ops = ps2.tile([128, 2 * S], f32, tag="ops", name="ops")
                            for mc in range(4):
                                nc.tensor.matmul(
                                    ops[:, :ncols],
                                    w2[:, mc, n * 128 : (n + 1) * 128],
                                    rh1[:, mc, :ncols],
                                    start=(mc == 0),
                                    stop=(mc == 3),
                                )
                            if n < 2 or (n == 2 and b == B - 1 and u == NU - 1):
                                nc.scalar.activation(
                                    rh2[:, n, :ncols], ops[:, :ncols], AF.Relu
                                )
                            else:
                                nc.vector.tensor_scalar(
                                    out=rh2[:, n, :ncols],
                                    in0=ops[:, :ncols],
                                    scalar1=0.0,
                                    scalar2=None,
                                    op0=OP.max,
                                )
                        # GEMM3: rh2 stationary, Wp moving (1-col matmuls)
                        g3 = ps3.tile([100, 4], f32, tag="g3", name="g3",
                                      padded_shape=[100, 512])
                        for c in range(2 * nil):
                            for n in range(4):
                                nc.tensor.matmul(
                                    g3[:, c : c + 1],
                                    rh2[:, n, c * 100 : (c + 1) * 100],
                                    wp[:, n : n + 1],
                                    start=(c == 0 and n == 0),
                                    stop=(c == 2 * nil - 1 and n == 3),
                                )
                        nc.scalar.activation(
                            sgout[:, u * 4 : u * 4 + 2 * nil], g3[:, 0 : 2 * nil],
                            AF.Sigmoid,
                        )
                        if u == 5:
                            nc.sync.dma_start(
                                dout[0:100, b * NCH : b * NCH + 24], sgout[:, 0:24])
                        if u == NU - 1:
                            nc.sync.dma_start(
                                dout[0:100, b * NCH + 24 : (b + 1) * NCH],
                                sgout[:, 24:NCH])

                for b in range(B):
                    for u in range(NU):
                        if b == B - 1 and u == 9:
                            # prebuild the final (short) unit's rh1 early so
                            # the kernel tail isn't gated on its DVE chain
                            prebuilt[(b, NU - 1)] = emit_rh1(b, NU - 1)
                        emit_unit(b, u)

    nc.compile()
    return nc


def _host_prep(x, pe_k, pe_v, comp, basis, root, rgcn_bias, W1, W2, Wp):
    import ml_dtypes

    bf16 = ml_dtypes.bfloat16

    ra = _rel_adj(S) % NREL
    onehot = (ra[None, :, :] == np.arange(NREL)[:, None, None]).astype(np.float64)
    deg = onehot.sum(1)
    inv = np.where(deg > 0, 1.0 / np.maximum(deg, 1.0), 0.0)
    anorm = onehot * inv[:, None, :]
    ahat_full = np.einsum("rk,rij->kij", np.asarray(comp, np.float64), anorm)
    ahat_full = ahat_full.astype(np.float32)  # [2, S, S]
    pos = np.clip(np.arange(S)[:, None] - np.arange(S)[None, :] + 1, 0, MAXL)

    x = np.asarray(x, np.float32)
    W1 = np.asarray(W1, np.float32)
    W1a, W1b = W1[:D], W1[D : D + P]
    W1c, W1d = W1[D + P : 2 * D + P], W1[2 * D + P :]

    # T table on host: [11, 512]
    T = (np.asarray(pe_k, np.float32) @ W1b + np.asarray(pe_v, np.float32) @ W1d)
    Tm = np.ascontiguousarray(T.T)  # [512, 11]

    com = {
        "basis": np.concatenate(
            [_pack_k(np.asarray(basis[k], np.float32)) for k in range(2)], axis=1
        ).astype(bf16),
        "root": _pack_k(np.asarray(root, np.float32)).astype(bf16),
        "w1a": _pack_k(W1a).astype(bf16),
        "w1c": _pack_k(W1c).astype(bf16),
        "w2": np.ascontiguousarray(
            np.asarray(W2, np.float32).reshape(4, 128, M)
            .transpose(1, 0, 2).reshape(128, 4 * M)).astype(bf16),
        "wp": np.ascontiguousarray(np.asarray(Wp, np.float32)[:, 0]
                                   .reshape(4, 128).T).astype(bf16),
    }
    bias_p = np.zeros((128, 3), np.float32)
    rb = np.asarray(rgcn_bias, np.float32)
    for c in range(3):
        r = min(128, D - c * 128)
        bias_p[:r, c] = rb[c * 128 : c * 128 + r]
    com["bias"] = bias_p

    xt_all = x.transpose(2, 0, 1)  # [D, B, S]
    per_core = []
    for c in range(NCORES):
        i0 = c * IPC
        m = dict(com)
        xtc = np.empty((D, B * SC), np.float32)
        for b in range(B):
            xtc[:, b * SC : b * SC + S] = xt_all[:, b, :]
            xtc[:, b * SC + S : (b + 1) * SC] = xt_all[:, b, i0 : i0 + IPC]
        m["xT"] = xtc.astype(bf16)
        ah = np.zeros((128, 4 * SC), np.float32)
        for k in range(2):
            for jc in range(2):
                r = 128 if jc == 0 else 72
                base = (k * 2 + jc) * SC
                ah[:r, base : base + S] = ahat_full[k, jc * 128 : jc * 128 + r, :]
                ah[:r, base + S : base + SC] = ahat_full[k, jc * 128 : jc * 128 + r, i0 : i0 + IPC]
        m["ahat"] = ah.astype(bf16)
        # peR: u-major blocks [128, NU*1600]; block u = [4 mc x 400 cols]
        pf = pos[i0 : i0 + IPC, :].reshape(-1)  # [5000]
        G = Tm[:, pf].reshape(4, 128, FPC)  # [mc, 128, 5000]
        pr = np.zeros((128, NU, 4, 400), np.float32)
        for u in range(NU):
            w = 400 if u < NU - 1 else 200
            pr[:, u, :, :w] = G[:, :, u * 400 : u * 400 + w].transpose(1, 0, 2)
        m["peR"] = np.ascontiguousarray(pr.reshape(128, NU * 1600)).astype(bf16)
        per_core.append(m)
    return per_core


def kernel(x, mask, pe_k, pe_v, comp, basis, root, rgcn_bias, W1, W2, Wp,
           _want_results=False, _trace=False):
    from concourse.bass_utils import run_bass_kernel_spmd

    if "nc" not in _prog_cache:
        _prog_cache["nc"] = _build_program()
    nc = _prog_cache["nc"]

    in_maps = _host_prep(x, pe_k, pe_v, comp, basis, root, rgcn_bias, W1, W2, Wp)
    res = run_bass_kernel_spmd(nc, in_maps, core_ids=list(range(NCORES)),
                               trace=_trace)

    out = np.zeros((B, S, S), np.float32)
    for c in range(NCORES):
        i0 = c * IPC
        r = res.results[c]["out"]  # [100, B*NCH]
        for b in range(B):
            blk = r[:, b * NCH : (b + 1) * NCH]  # [100, 50], pair = col*100 + p
            out[b, i0 : i0 + IPC, :] = blk.T.reshape(IPC, S)
    out *= np.asarray(mask, np.float32)
    if _want_results:
        return out, res
    return out
